# revision 15
# baseline (speedup 1.0000x reference)
"""DeltaTokenShift Trainium2 kernel (Bass/Tile, 8 NeuronCores via axon).

Computation (per batch b):
    erase = sigmoid(x @ We + be) ; write = sigmoid(x @ Ww + bw)
    s_t = s_{t-1} * (1 - erase_t) + write_t * x_t   (scan over L, per channel)
    out[:, t, :] = s_t

Sharding: 8 cores = 4 batches x 2 halves of the 1024-channel dim. Each core
gets the full x[b] (the gate matmul contracts over all 1024 input channels),
its 512-column weight slices, bias/state slices, and computes
out[b][:, half] = [4096, 512]. For upper-half cores, x columns and weight
rows are rotated by 512 on the host so the core's own gate channels always
occupy xT k-tiles 0..3 (a consistent permutation of the contraction dim
leaves the matmul result unchanged).

All layout changes are done host-side: x ships pre-transposed as
xT = [1024 d, 4096 l] (so no PE transposes are needed to put the
contraction dim on partitions), and the kernel writes outT = [512 e,
4096 l], transposed back on the host. The PE then runs ONLY the 512 gate
matmuls (f32r, 1 col/cycle), which is the roofline for this op at fp32
precision. xT tiles are DMA'd once as f32r and bitcast to f32 for the
scan's b-term (f32r and f32 share the bit layout).

Per-core pipeline over 1024-token DMA blocks (2x 512-token compute chunks):
  DMA xT k-slabs [128, 1024] -> for each 512 chunk, per m-group:
  8-step f32r matmul accumulation [128e, 512l] in PSUM for each gate,
  ACT sigmoid straight from PSUM (erase uses scale=-1, bias=-be =>
  a = 1-sigmoid), GpSimd b = write * xT, DVE tensor_tensor_scan(a, b)
  chained via initial=prev[:, -1:], contiguous DMA of [128, 1024] out.
"""

import sys

sys.path.insert(0, "/opt/trn_rl_repo")

import numpy as np
import concourse.bacc as bacc
import concourse.mybir as mybir
from concourse.tile import TileContext
from concourse.bass_utils import run_bass_kernel_spmd

B, L = 4, 4096

F32 = mybir.dt.float32
F32R = mybir.dt.float32r
BF16 = mybir.dt.bfloat16

P = 128
DIN = 1024
ESH = 512
KT = DIN // P  # 8 contraction k-tiles
MT = ESH // P  # 4 output-channel groups per core


def _build_kernel_impl(L=4096, blocks=None, wdt="f32r", xdt="f32r", warmup=0,
                       w_queue="sync", x_queue="sync", o_queue="sync"):
    """blocks: list of lists of chunk widths; each inner list is one DMA
    block (xT slab + out slab)."""
    if blocks is None:
        blocks = [[512, 512]] * (L // 1024)
    assert sum(sum(b) for b in blocks) == L
    lbmax = max(sum(b) for b in blocks)
    wmdt = {"f32r": F32R, "bf16": BF16}[wdt]
    xmdt = {"f32r": F32R, "bf16": BF16}[xdt]

    nc = bacc.Bacc("TRN2", target_bir_lowering=False)
    queues = {"sync": nc.sync, "scalar": nc.scalar, "vector": nc.vector,
              "gpsimd": nc.gpsimd}
    wq, xq, oq = queues[w_queue], queues[x_queue], queues[o_queue]

    xT = nc.dram_tensor("xT", [DIN, L], xmdt, kind="ExternalInput")
    we = nc.dram_tensor("we", [DIN, ESH], wmdt, kind="ExternalInput")
    ww = nc.dram_tensor("ww", [DIN, ESH], wmdt, kind="ExternalInput")
    # biases[:, m] = -erase_bias group m ; biases[:, MT+m] = +write_bias group m
    biases = nc.dram_tensor("biases", [P, 2 * MT], F32, kind="ExternalInput")
    state0 = nc.dram_tensor("state0", [P, MT], F32, kind="ExternalInput")
    outT = nc.dram_tensor("outT", [ESH, L], F32, kind="ExternalOutput")

    with TileContext(nc) as tc:
        with (
            tc.tile_pool(name="const", bufs=1) as constp,
            tc.tile_pool(name="wsb", bufs=1) as wsb,
            tc.tile_pool(name="xt", bufs=2) as xtp,
            tc.tile_pool(name="gate", bufs=4) as gatep,
            tc.tile_pool(name="bmul", bufs=3) as bmulp,
            tc.tile_pool(name="scan", bufs=2) as scanp,
            tc.tile_pool(name="ps_mm", bufs=4, space="PSUM") as ps_mm,
        ):
            if warmup:
                # Dummy matmuls on a zeroed tile: keeps the PE busy while
                # the first DMAs land so the DVFS clock is already ramped
                # to full speed when real matmuls start.
                wu_sb = constp.tile([P, ESH], BF16 if xdt == "bf16" else F32,
                                    tag="wu")
                nc.gpsimd.memset(wu_sb[:], 0.0)
                wu_ps = [ps_mm.tile([P, ESH], F32, tag=f"wups{j}",
                                      name=f"wups{j}", bufs=1)
                         for j in range(2)]
                wu_l = wu_sb[:, :P] if xdt == "bf16" else \
                    wu_sb[:, :P].bitcast(F32R)
                wu_r = wu_sb[:] if xdt == "bf16" else wu_sb[:].bitcast(F32R)
                for j in range(warmup):
                    nc.tensor.matmul(
                        wu_ps[j % 2][:], wu_l, wu_r,
                        start=True, stop=True, skip_group_check=True)

            # Interleave weight and block-0 xT DMAs so the PE can trickle
            # through the first chunk's k-accumulation while later k-tiles
            # are still streaming in.
            lb0 = sum(blocks[0])
            w_tiles = [[None] * KT for _ in range(2)]
            xt0 = [None] * KT
            for k in range(KT):
                for gi, wt in enumerate((we, ww)):
                    t = wsb.tile([P, ESH], wmdt, tag=f"w{gi}_{k}")
                    wq.dma_start(t[:], wt[k * P:(k + 1) * P, :])
                    w_tiles[gi][k] = t
                t = xtp.tile([P, lbmax], xmdt, tag=f"xt{k}", name=f"xt{k}")
                xq.dma_start(t[:, :lb0], xT[k * P:(k + 1) * P, :lb0])
                xt0[k] = t

            bias_sb = constp.tile([P, 2 * MT], F32, tag="bias")
            nc.sync.dma_start(bias_sb[:], biases[:])
            st_sb = constp.tile([P, MT], F32, tag="st")
            nc.sync.dma_start(st_sb[:], state0[:])

            prev_s = [None] * MT
            b0 = 0

            for blki, chunks in enumerate(blocks):
                lb = sum(chunks)
                if blki == 0:
                    xt = xt0
                else:
                    xt = []
                    for k in range(KT):
                        t = xtp.tile([P, lbmax], xmdt, tag=f"xt{k}",
                                     name=f"xt{k}")
                        xq.dma_start(
                            t[:, :lb], xT[k * P:(k + 1) * P, b0:b0 + lb])
                        xt.append(t)

                # Hoist the f32 casts of the b-term x slabs out of the
                # per-m critical chain: they only depend on the xT DMAs.
                xf = [None] * MT
                if xdt == "bf16":
                    for m in range(MT):
                        xf[m] = bmulp.tile([P, lbmax], F32, tag=f"xf{m}",
                                           name=f"xf{m}", bufs=2)
                        nc.vector.tensor_copy(xf[m][:, :lb], xt[m][:, :lb])

                def mm_group(gi, m, lo, lc):
                    ps = ps_mm.tile([P, ESH], F32, tag="psmm", name="psmm")
                    for k in range(KT):
                        nc.tensor.matmul(
                            ps[:, :lc],
                            w_tiles[gi][k][:, m * P:(m + 1) * P],
                            xt[k][:, lo:lo + lc],
                            start=(k == 0), stop=(k == KT - 1),
                        )
                    g_t = gatep.tile([P, ESH], F32, tag="aw"[gi],
                                     name="aw"[gi])
                    nc.scalar.activation(
                        g_t[:, :lc], ps[:, :lc],
                        mybir.ActivationFunctionType.Sigmoid,
                        bias=bias_sb[:, gi * MT + m:gi * MT + m + 1],
                        scale=-1.0 if gi == 0 else 1.0,
                    )
                    return g_t

                s_tiles = [None] * MT
                lo = 0
                for ci, lc in enumerate(chunks):
                    gates = [[None] * MT, [None] * MT]
                    if blki == 0 and ci == 0:
                        # Gate-major: the erase groups only need the we
                        # tiles, which land first during the preload.
                        for gi in range(2):
                            for m in range(MT):
                                gates[gi][m] = mm_group(gi, m, lo, lc)
                    else:
                        for m in range(MT):
                            gates[0][m] = mm_group(0, m, lo, lc)
                            gates[1][m] = mm_group(1, m, lo, lc)

                    for m in range(MT):
                        a_t, w_t = gates[0][m], gates[1][m]
                        x_op = xf[m][:, lo:lo + lc] if xdt == "bf16" \
                            else xt[m][:, lo:lo + lc].bitcast(F32)
                        b_t = bmulp.tile([P, ESH], F32, tag="b")
                        # GpSimd is otherwise idle; fully parallel with DVE,
                        # and both operands + out are SBUF (P2-safe).
                        nc.gpsimd.tensor_tensor(
                            b_t[:, :lc], w_t[:, :lc], x_op,
                            op=mybir.AluOpType.mult)

                        if ci == 0:
                            s_tiles[m] = scanp.tile(
                                [P, lbmax], F32, tag=f"s{m}", name=f"s{m}")
                            init = st_sb[:, m:m + 1] if blki == 0 else \
                                prev_s[m][:, prev_lb - 1:prev_lb]
                        else:
                            init = s_tiles[m][:, lo - 1:lo]
                        nc.vector.tensor_tensor_scan(
                            s_tiles[m][:, lo:lo + lc], a_t[:, :lc],
                            b_t[:, :lc], init,
                            op0=mybir.AluOpType.mult, op1=mybir.AluOpType.add,
                        )
                        if ci == len(chunks) - 1:
                            oq.dma_start(
                                outT[m * P:(m + 1) * P, b0:b0 + lb],
                                s_tiles[m][:, :lb])
                            prev_s[m] = s_tiles[m]
                    lo += lc
                prev_lb = lb
                b0 += lb

    nc.finalize()
    return nc


_cached_nc = None
_WDT = "bf16"
_XDT = "bf16"


def _build_kernel():
    blocks = [[512], [512, 512], [512, 512], [512, 512], [512]]
    return _build_kernel_impl(L=L, blocks=blocks, wdt=_WDT, xdt=_XDT,
                              warmup=12,
                              w_queue="scalar", x_queue="sync",
                              o_queue="gpsimd")


def _shard_inputs(x, state, erase_kernel, erase_bias, write_kernel, write_bias):
    try:
        import ml_dtypes
        bf16 = ml_dtypes.bfloat16
    except ImportError:
        bf16 = None
    maps = []
    for core in range(8):
        b, h = divmod(core, 2)
        e0 = h * ESH
        xb = x[b]
        web = erase_kernel[:, e0:e0 + ESH]
        wwb = write_kernel[:, e0:e0 + ESH]
        if h == 1:
            xb = np.concatenate([xb[:, ESH:], xb[:, :ESH]], axis=1)
            web = np.concatenate([web[ESH:, :], web[:ESH, :]], axis=0)
            wwb = np.concatenate([wwb[ESH:, :], wwb[:ESH, :]], axis=0)
        if _WDT == "bf16":
            web = web.astype(bf16)
            wwb = wwb.astype(bf16)
        xbT = np.ascontiguousarray(xb.T)
        if _XDT == "bf16":
            xbT = xbT.astype(bf16)
        ben = (-erase_bias[e0:e0 + ESH]).reshape(MT, P).T
        bwp = write_bias[e0:e0 + ESH].reshape(MT, P).T
        stp = state[b, e0:e0 + ESH].reshape(MT, P).T
        maps.append({
            "xT": xbT,
            "we": np.ascontiguousarray(web),
            "ww": np.ascontiguousarray(wwb),
            "biases": np.ascontiguousarray(
                np.concatenate([ben, bwp], axis=1), dtype=np.float32),
            "state0": np.ascontiguousarray(stp, dtype=np.float32),
        })
    return maps


def kernel(x, state, erase_kernel, erase_bias, write_kernel, write_bias):
    global _cached_nc
    x = np.asarray(x, np.float32)
    state = np.asarray(state, np.float32)
    erase_kernel = np.asarray(erase_kernel, np.float32)
    erase_bias = np.asarray(erase_bias, np.float32)
    write_kernel = np.asarray(write_kernel, np.float32)
    write_bias = np.asarray(write_bias, np.float32)

    if _cached_nc is None:
        _cached_nc = _build_kernel()
    maps = _shard_inputs(x, state, erase_kernel, erase_bias,
                         write_kernel, write_bias)
    res = run_bass_kernel_spmd(_cached_nc, maps, core_ids=list(range(8)))
    full = np.empty((B, L, DIN), np.float32)
    for core in range(8):
        b, h = divmod(core, 2)
        full[b, :, h * ESH:(h + 1) * ESH] = res.results[core]["outT"].T
    return full


# revision 19
# speedup vs baseline: 1.0564x; 1.0564x over previous
"""DeltaTokenShift Trainium2 kernel (Bass/Tile, 8 NeuronCores via axon).

Computation (per batch b):
    erase = sigmoid(x @ We + be) ; write = sigmoid(x @ Ww + bw)
    s_t = s_{t-1} * (1 - erase_t) + write_t * x_t   (scan over L, per channel)
    out[:, t, :] = s_t

Sharding: 8 cores = 4 batches x 2 halves of the 1024-channel dim. Each core
gets the full x[b] (the gate matmul contracts over all 1024 input channels),
its 512-column weight slices, bias/state slices, and computes
out[b][:, half] = [4096, 512]. For upper-half cores, x columns and weight
rows are rotated by 512 on the host so the core's own gate channels always
occupy xT k-tiles 0..3 (a consistent permutation of the contraction dim
leaves the matmul result unchanged).

All layout work is host-side. x ships transposed, bf16, and packed in
k-planes: xT[p, k, l] = x[l, 128k+p] (so the contraction dim sits on
partitions with no PE transposes, and one DMA issue can move any k-range).
Weights likewise pack to [128, k, e] bf16; the kernel writes
outT[p, m, l] = out[l, 128m+p], unpacked on the host. bf16 gate inputs
keep the PE at 1 col/cycle (same as f32r) while halving HBM traffic;
fp8 was measured out of tolerance (2.6e-2) and bf16 in (3.3e-3).

The PE then runs ONLY the 512 gate matmuls [128e, 512l] = 8-step bf16
accumulations in PSUM, which is the compute floor for this op. Per
512-token chunk, per m-group: ACT sigmoid straight from PSUM (erase uses
scale=-1, bias=-be => a = 1-sigmoid), GpSimd b = write * x_f32 (x cast
hoisted to block top on DVE), DVE tensor_tensor_scan(a, b) chained via
initial=prev[:, -1:], per-m DMA of outT block slabs.
"""

import sys

sys.path.insert(0, "/opt/trn_rl_repo")

import numpy as np
import concourse.bacc as bacc
import concourse.mybir as mybir
from concourse.tile import TileContext
from concourse.bass_utils import run_bass_kernel_spmd

B, L = 4, 4096

F32 = mybir.dt.float32
F32R = mybir.dt.float32r
BF16 = mybir.dt.bfloat16

P = 128
DIN = 1024
ESH = 512
KT = DIN // P  # 8 contraction k-tiles
MT = ESH // P  # 4 output-channel groups per core


def _build_kernel_impl(L=4096, blocks=None, warmup=0, psum_bufs=4,
                       preload_groups=((0,), (1,), (2, 3), (4, 5, 6, 7)),
                       w_queue="sync", x_queue="sync", o_queue="sync"):
    """blocks: list of lists of chunk widths; each inner list is one DMA
    block (xT slab in, outT slab out). preload_groups: k-ranges, one DMA
    issue each, for the weight + block-0 x preload."""
    if blocks is None:
        blocks = [[512], [512, 512], [512, 512], [512, 512], [512]]
    assert sum(sum(b) for b in blocks) == L
    lbmax = max(sum(b) for b in blocks)
    assert sorted(k for g in preload_groups for k in g) == list(range(KT))
    for g in preload_groups:
        assert list(g) == list(range(g[0], g[0] + len(g)))

    nc = bacc.Bacc("TRN2", target_bir_lowering=False)
    queues = {"sync": nc.sync, "scalar": nc.scalar, "vector": nc.vector,
              "gpsimd": nc.gpsimd}
    wq, xq, oq = queues[w_queue], queues[x_queue], queues[o_queue]
    wu_rand = nc.dram_tensor("wu_rand", [P, ESH], BF16,
                             kind="ExternalInput")

    xT = nc.dram_tensor("xT", [P, KT, L], BF16, kind="ExternalInput")
    we = nc.dram_tensor("we", [P, KT, ESH], BF16, kind="ExternalInput")
    ww = nc.dram_tensor("ww", [P, KT, ESH], BF16, kind="ExternalInput")
    # biases[:, m] = -erase_bias group m ; biases[:, MT+m] = +write_bias group m
    biases = nc.dram_tensor("biases", [P, 2 * MT], F32, kind="ExternalInput")
    state0 = nc.dram_tensor("state0", [P, MT], F32, kind="ExternalInput")
    outT = nc.dram_tensor("outT", [P, MT, L], F32, kind="ExternalOutput")

    with TileContext(nc) as tc:
        with (
            tc.tile_pool(name="const", bufs=1) as constp,
            tc.tile_pool(name="wsb", bufs=1) as wsb,
            tc.tile_pool(name="xt", bufs=2) as xtp,
            tc.tile_pool(name="gate", bufs=4) as gatep,
            tc.tile_pool(name="bmul", bufs=3) as bmulp,
            tc.tile_pool(name="scan", bufs=2) as scanp,
            tc.tile_pool(name="ps_mm", bufs=psum_bufs, space="PSUM") as ps_mm,
        ):
            if warmup:
                # Dummy matmuls keep the PE busy while the first DMAs
                # land. Random operand data (not zeros): the DVFS governor
                # responds to switching activity, so zero-valued warmups
                # leave the clock low.
                wu_sb = constp.tile([P, ESH], BF16, tag="wu")
                nc.sync.dma_start(wu_sb[:], wu_rand[:])
                wu_ps = [ps_mm.tile([P, ESH], F32, tag=f"wups{j}",
                                    name=f"wups{j}", bufs=1)
                         for j in range(2)]
                for j in range(warmup):
                    nc.tensor.matmul(
                        wu_ps[j % 2][:], wu_sb[:, :P], wu_sb[:],
                        start=True, stop=True, skip_group_check=True)

            # Preload: erase weights + block-0 x first (the first chunk is
            # emitted gate-major, so the PE only needs we+x to start), then
            # the write weights, which land during the erase sweep.
            lb0 = sum(blocks[0])
            w_sb = [wsb.tile([P, KT, ESH], BF16, tag=f"w{gi}", name=f"w{gi}")
                    for gi in range(2)]
            xt0 = xtp.tile([P, KT, lbmax], BF16, tag="xt", name="xt")
            for g in preload_groups:
                ks = slice(g[0], g[-1] + 1)
                wq.dma_start(w_sb[0][:, ks, :], we[:, ks, :])
                xq.dma_start(xt0[:, ks, :lb0], xT[:, ks, :lb0])
            for g in preload_groups:
                ks = slice(g[0], g[-1] + 1)
                wq.dma_start(w_sb[1][:, ks, :], ww[:, ks, :])

            bias_sb = constp.tile([P, 2 * MT], F32, tag="bias")
            nc.sync.dma_start(bias_sb[:], biases[:])
            st_sb = constp.tile([P, MT], F32, tag="st")
            nc.sync.dma_start(st_sb[:], state0[:])

            prev_s = [None] * MT
            b0 = 0

            for blki, chunks in enumerate(blocks):
                lb = sum(chunks)
                if blki == 0:
                    xt = xt0
                else:
                    xt = xtp.tile([P, KT, lbmax], BF16, tag="xt", name="xt")
                    xq.dma_start(xt[:, :, :lb], xT[:, :, b0:b0 + lb])

                # Hoist the f32 casts of the b-term x slabs out of the
                # per-m critical chain: they only depend on the xT DMA.
                xf = [None] * MT
                for m in range(MT):
                    xf[m] = bmulp.tile([P, lbmax], F32, tag=f"xf{m}",
                                       name=f"xf{m}", bufs=2)
                    nc.vector.tensor_copy(xf[m][:, :lb], xt[:, m, :lb])

                def mm_group(gi, m, lo, lc):
                    ps = ps_mm.tile([P, ESH], F32, tag="psmm", name="psmm")
                    for k in range(KT):
                        nc.tensor.matmul(
                            ps[:, :lc],
                            w_sb[gi][:, k, m * P:(m + 1) * P],
                            xt[:, k, lo:lo + lc],
                            start=(k == 0), stop=(k == KT - 1),
                        )
                    g_t = gatep.tile([P, ESH], F32, tag="aw"[gi],
                                     name="aw"[gi])
                    nc.scalar.activation(
                        g_t[:, :lc], ps[:, :lc],
                        mybir.ActivationFunctionType.Sigmoid,
                        bias=bias_sb[:, gi * MT + m:gi * MT + m + 1],
                        scale=-1.0 if gi == 0 else 1.0,
                    )
                    return g_t

                s_tiles = [None] * MT
                lo = 0
                for ci, lc in enumerate(chunks):
                    gates = [[None] * MT, [None] * MT]
                    if blki == 0 and ci == 0:
                        # Gate-major: the erase groups only need the we
                        # tiles, which land first during the preload.
                        for gi in range(2):
                            for m in range(MT):
                                gates[gi][m] = mm_group(gi, m, lo, lc)
                    else:
                        for m in range(MT):
                            gates[0][m] = mm_group(0, m, lo, lc)
                            gates[1][m] = mm_group(1, m, lo, lc)

                    for m in range(MT):
                        a_t, w_t = gates[0][m], gates[1][m]
                        b_t = bmulp.tile([P, ESH], F32, tag="b")
                        # GpSimd is otherwise idle; fully parallel with DVE,
                        # and both operands + out are SBUF (P2-safe).
                        nc.gpsimd.tensor_tensor(
                            b_t[:, :lc], w_t[:, :lc], xf[m][:, lo:lo + lc],
                            op=mybir.AluOpType.mult)

                        if ci == 0:
                            s_tiles[m] = scanp.tile(
                                [P, lbmax], F32, tag=f"s{m}", name=f"s{m}")
                            init = st_sb[:, m:m + 1] if blki == 0 else \
                                prev_s[m][:, prev_lb - 1:prev_lb]
                        else:
                            init = s_tiles[m][:, lo - 1:lo]
                        nc.vector.tensor_tensor_scan(
                            s_tiles[m][:, lo:lo + lc], a_t[:, :lc],
                            b_t[:, :lc], init,
                            op0=mybir.AluOpType.mult, op1=mybir.AluOpType.add,
                        )
                        if ci == len(chunks) - 1:
                            oq.dma_start(
                                outT[:, m, b0:b0 + lb], s_tiles[m][:, :lb])
                            prev_s[m] = s_tiles[m]
                    lo += lc
                prev_lb = lb
                b0 += lb

    nc.finalize()
    return nc


_cached_nc = None


def _build_kernel():
    return _build_kernel_impl(
        L=L, warmup=8, preload_groups=((0, 1, 2, 3), (4, 5, 6, 7)))


def _shard_inputs(x, state, erase_kernel, erase_bias, write_kernel, write_bias):
    import ml_dtypes
    bf16 = ml_dtypes.bfloat16

    def pack_k(a2d):  # [DIN, C] -> [P, KT, C]
        return np.ascontiguousarray(
            a2d.reshape(KT, P, a2d.shape[1]).transpose(1, 0, 2))

    maps = []
    for core in range(8):
        b, h = divmod(core, 2)
        e0 = h * ESH
        xb = x[b]
        web = erase_kernel[:, e0:e0 + ESH]
        wwb = write_kernel[:, e0:e0 + ESH]
        if h == 1:
            xb = np.concatenate([xb[:, ESH:], xb[:, :ESH]], axis=1)
            web = np.concatenate([web[ESH:, :], web[:ESH, :]], axis=0)
            wwb = np.concatenate([wwb[ESH:, :], wwb[:ESH, :]], axis=0)
        ben = (-erase_bias[e0:e0 + ESH]).reshape(MT, P).T
        bwp = write_bias[e0:e0 + ESH].reshape(MT, P).T
        stp = state[b, e0:e0 + ESH].reshape(MT, P).T
        maps.append({
            "wu_rand": np.random.RandomState(0).standard_normal(
                (P, ESH)).astype(bf16),
            "xT": pack_k(np.ascontiguousarray(xb.T).astype(bf16)),
            "we": pack_k(web.astype(bf16)),
            "ww": pack_k(wwb.astype(bf16)),
            "biases": np.ascontiguousarray(
                np.concatenate([ben, bwp], axis=1), dtype=np.float32),
            "state0": np.ascontiguousarray(stp, dtype=np.float32),
        })
    return maps


def kernel(x, state, erase_kernel, erase_bias, write_kernel, write_bias):
    global _cached_nc
    x = np.asarray(x, np.float32)
    state = np.asarray(state, np.float32)
    erase_kernel = np.asarray(erase_kernel, np.float32)
    erase_bias = np.asarray(erase_bias, np.float32)
    write_kernel = np.asarray(write_kernel, np.float32)
    write_bias = np.asarray(write_bias, np.float32)

    if _cached_nc is None:
        _cached_nc = _build_kernel()
    maps = _shard_inputs(x, state, erase_kernel, erase_bias,
                         write_kernel, write_bias)
    res = run_bass_kernel_spmd(_cached_nc, maps, core_ids=list(range(8)))
    full = np.empty((B, L, DIN), np.float32)
    for core in range(8):
        b, h = divmod(core, 2)
        o = res.results[core]["outT"]  # [P, MT, L]
        full[b, :, h * ESH:(h + 1) * ESH] = \
            o.transpose(2, 1, 0).reshape(L, ESH)
    return full
